# revision 1
# baseline (speedup 1.0000x reference)
"""Windowed sparse attention kernel for TRN2 (8 NeuronCores).

Problem: b=1, h=16, n=16384, d=32, window w=128, nw=128 windows.
Each window of 128 queries attends to [4 memory slots | prev window | cur window]
with additive bias, tanh softcap (50), softmax.

Sharding: sequence-parallel over windows. Core c handles windows
[c*16, (c+1)*16) for all 16 heads, with a one-window k/v halo.

Device dataflow (keys on partitions, slot-major):
  For k/v slot s (17 per core), one N=256 matmul computes
  simT[key_s, (q_{s-1} | q_s)] — slot s's keys against both query windows
  that attend to it (cur role for task s-1, prev role for task s).
  DVE adds the (pre-arranged, mask-folded) bias while evacuating PSUM.
  ACT applies tanh softcap + exp in wide 4352-col instructions.
  mm2 flips orientation: lhsT = p-slice (keys x queries), rhs = v~ (keys x 33)
  -> out (128 q, 33) per task, where v~'s ones column makes col 32 the
  softmax denominator Z. Host combines the 4-slot memory attention
  (1.5% of keys) and normalizes.
"""

import numpy as np

B, H, N, D = 1, 16, 16384, 32
W = 128                 # window size
NW = N // W             # 128 windows
NCORES = 8
WPC = NW // NCORES      # 16 windows (tasks) per core
NSLOT = WPC + 1         # 17 k/v slots (halo)
SOFTCLAMP = 50.0
SCALE = D ** -0.5
MASK_PEN = -30000.0
SIMW = NSLOT * 256      # 4352 wide-tile cols (slot-major, 256 per slot)

_COMPILED = None


def _build_bass():
    import concourse.bacc as bacc
    import concourse.tile as tile
    from concourse import mybir
    from contextlib import ExitStack

    f32 = mybir.dt.float32
    nc = bacc.Bacc()

    qT = nc.declare_dram_parameter("qT", [4, 128, WPC * W], f32, isOutput=False)
    kT = nc.declare_dram_parameter("kT", [4, 128, NSLOT * W], f32, isOutput=False)
    vv = nc.declare_dram_parameter("vv", [H, 128, NSLOT * 33], f32, isOutput=False)
    bT = nc.declare_dram_parameter("bT", [128, SIMW], f32, isOutput=False)
    o = nc.declare_dram_parameter("o", [H, 128, WPC * 33], f32, isOutput=True)

    with ExitStack() as ctx:
        tc = ctx.enter_context(tile.TileContext(nc))
        singles = ctx.enter_context(tc.tile_pool(name="singles", bufs=1))
        qk_pool = ctx.enter_context(tc.tile_pool(name="qk", bufs=2))
        v_pool = ctx.enter_context(tc.tile_pool(name="v", bufs=2))
        wide = ctx.enter_context(tc.tile_pool(name="wide", bufs=2))
        ow_pool = ctx.enter_context(tc.tile_pool(name="ow", bufs=2))
        sim_ps = ctx.enter_context(tc.tile_pool(name="simps", bufs=4, space="PSUM"))
        out_ps = ctx.enter_context(tc.tile_pool(name="outps", bufs=4, space="PSUM"))

        biasS = singles.tile([128, SIMW], f32)
        nc.sync.dma_start(out=biasS[:, :], in_=bT[:, :])

        for g in range(4):
            Qg = qk_pool.tile([128, WPC * W], f32, tag="qg")
            nc.sync.dma_start(out=Qg[:, :], in_=qT[g])
            Kg = qk_pool.tile([128, NSLOT * W], f32, tag="kg")
            nc.sync.dma_start(out=Kg[:, :], in_=kT[g])
            for i in range(4):
                h = 4 * g + i
                p0 = 32 * i
                Vh = v_pool.tile([128, NSLOT * 33], f32)
                nc.sync.dma_start(out=Vh[:, :], in_=vv[h])

                simS = wide.tile([128, SIMW], f32, tag="simS")
                # mm1: per slot-pair bank; slot s block = [cur(q_{s-1}) | prev(q_s)]
                for b in range(9):
                    ncols = 512 if b < 8 else 256
                    simP = sim_ps.tile([128, 512], f32)
                    for sub in range(2 if b < 8 else 1):
                        s = 2 * b + sub
                        off = sub * 256
                        lhsT = Kg[p0:p0 + 32, s * W:(s + 1) * W]
                        if s == 0:
                            # filler for nonexistent task -1 (finite, unused)
                            nc.tensor.matmul(simP[:, 0:128], lhsT=lhsT,
                                             rhs=Qg[p0:p0 + 32, 0:W],
                                             start=True, stop=True,
                                             tile_position=(p0, 0))
                            nc.tensor.matmul(simP[:, 128:256], lhsT=lhsT,
                                             rhs=Qg[p0:p0 + 32, 0:W],
                                             start=True, stop=True,
                                             tile_position=(p0, 0))
                        elif s == NSLOT - 1:
                            nc.tensor.matmul(simP[:, off:off + 128], lhsT=lhsT,
                                             rhs=Qg[p0:p0 + 32, (s - 1) * W:s * W],
                                             start=True, stop=True,
                                             tile_position=(p0, 0))
                            nc.tensor.matmul(simP[:, off + 128:off + 256], lhsT=lhsT,
                                             rhs=Qg[p0:p0 + 32, (s - 1) * W:s * W],
                                             start=True, stop=True,
                                             tile_position=(p0, 0))
                        else:
                            nc.tensor.matmul(simP[:, off:off + 256], lhsT=lhsT,
                                             rhs=Qg[p0:p0 + 32, (s - 1) * W:(s + 1) * W],
                                             start=True, stop=True,
                                             tile_position=(p0, 0))
                    nc.vector.tensor_add(
                        simS[:, b * 512:b * 512 + ncols],
                        simP[:, 0:ncols],
                        biasS[:, b * 512:b * 512 + ncols],
                    )
                # softcap + exp, wide
                tS = wide.tile([128, SIMW], f32, tag="tS")
                nc.scalar.activation(tS[:, :], simS[:, :],
                                     mybir.ActivationFunctionType.Tanh,
                                     scale=1.0 / SOFTCLAMP)
                pS = wide.tile([128, SIMW], f32, tag="pS")
                nc.scalar.activation(pS[:, :], tS[:, :],
                                     mybir.ActivationFunctionType.Exp,
                                     scale=SOFTCLAMP)
                # mm2: out (128 q, 33) per task, 8 tasks per PSUM bank
                outW = ow_pool.tile([128, WPC * 33], f32)
                for tb in range(2):
                    otP = out_ps.tile([128, 8 * 33], f32)
                    for u in range(8):
                        t = 8 * tb + u
                        # prev: slot t keys, q_t = second half of slot t block
                        nc.tensor.matmul(
                            otP[:, u * 33:(u + 1) * 33],
                            lhsT=pS[:, t * 256 + 128:t * 256 + 256],
                            rhs=Vh[:, t * 33:(t + 1) * 33],
                            start=True, stop=False)
                        # cur: slot t+1 keys, q_t = first half of slot t+1 block
                        nc.tensor.matmul(
                            otP[:, u * 33:(u + 1) * 33],
                            lhsT=pS[:, (t + 1) * 256:(t + 1) * 256 + 128],
                            rhs=Vh[:, (t + 1) * 33:(t + 2) * 33],
                            start=False, stop=True)
                    nc.vector.tensor_copy(outW[:, tb * 264:(tb + 1) * 264], otP[:, :])
                nc.sync.dma_start(out=o[h], in_=outW[:, :])
    nc.compile()
    return nc


def _get_compiled():
    global _COMPILED
    if _COMPILED is None:
        _COMPILED = _build_bass()
    return _COMPILED


def _prep_core(c, qs, ks, vs, ab, mvec):
    """Build per-core input arrays. qs,ks,vs: (H, N, D) (qs pre-scaled)."""
    w0 = c * WPC
    qw = qs.reshape(H, NW, W, D)[:, w0:w0 + WPC]          # (H,16,128,32)
    qTc = np.ascontiguousarray(
        qw.reshape(4, 4, WPC, W, D).transpose(0, 1, 4, 2, 3).reshape(4, 128, WPC * W))

    kw = ks.reshape(H, NW, W, D)
    vw = vs.reshape(H, NW, W, D)
    khalo = np.zeros((H, NSLOT, W, D), np.float32)
    vhalo = np.zeros((H, NSLOT, W, D), np.float32)
    lo = w0 - 1
    src_lo = max(lo, 0)
    dst_lo = src_lo - lo
    khalo[:, dst_lo:] = kw[:, src_lo:w0 + WPC]
    vhalo[:, dst_lo:] = vw[:, src_lo:w0 + WPC]
    kTc = np.ascontiguousarray(
        khalo.reshape(4, 4, NSLOT, W, D).transpose(0, 1, 4, 2, 3).reshape(4, 128, NSLOT * W))
    vvc = np.concatenate([vhalo, np.ones((H, NSLOT, W, 1), np.float32)], axis=3)
    vvc = np.ascontiguousarray(
        vvc.transpose(0, 2, 1, 3).reshape(H, 128, NSLOT * 33))

    # bias, slot-major: slot s block cols = [cur-bias(task s-1) | prev-bias(task s)]
    # both halves use keys of global window w0+s-1; fold key mask (+ structural
    # masking of window -1) as additive penalty.
    bTc = np.zeros((128, NSLOT, 2, W), np.float32)         # (key, slot, half, q)
    for s in range(NSLOT):
        gw = w0 + s - 1
        if s > 0:
            bTc[:, s, 0, :] = ab[gw, :, 128:256].T          # cur role for task s-1
        if s < NSLOT - 1:
            bTc[:, s, 1, :] = ab[gw + 1, :, 0:128].T        # prev role for task s
        if gw < 0:
            pen = np.full((W,), MASK_PEN, np.float32)
        else:
            pen = np.where(mvec[gw * W:(gw + 1) * W], np.float32(0),
                           np.float32(MASK_PEN))
        bTc[:, s, :, :] += pen[:, None, None]
    bTc = np.ascontiguousarray(bTc.reshape(128, SIMW))
    return {"qT": qTc, "kT": kTc, "vv": vvc, "bT": bTc}


def _run_device(in_maps, trace=False):
    from concourse.bass_utils import run_bass_kernel_spmd
    nc = _get_compiled()
    res = run_bass_kernel_spmd(nc, in_maps, list(range(NCORES)), trace=trace)
    return res


def kernel(q, k, v, mask, attn_bias, memory_kv, _trace=False, _ret_res=False):
    q = np.asarray(q, np.float32)
    k = np.asarray(k, np.float32)
    v = np.asarray(v, np.float32)
    mask = np.asarray(mask)
    attn_bias = np.asarray(attn_bias, np.float32)
    memory_kv = np.asarray(memory_kv, np.float32)

    qs = q[0] * np.float32(SCALE)       # (H, N, D)
    ks, vs = k[0], v[0]
    ab = attn_bias[0]                   # (NW, W, 2W)
    mvec = mask[0].astype(bool)         # (N,)

    in_maps = [_prep_core(c, qs, ks, vs, ab, mvec) for c in range(NCORES)]
    res = _run_device(in_maps, trace=_trace)
    outs = [r["o"] for r in res.results]             # each (H, 128, WPC*33)

    big = np.stack(outs)                              # (8, H, 128, 528)
    # (core, h, q, task, 33) -> (h, core, task, q, 33) -> (h, n, 33)
    arr = big.reshape(NCORES, H, W, WPC, 33).transpose(1, 0, 3, 2, 4)
    arr = arr.reshape(H, N, 33)
    num = arr[..., :D].astype(np.float64)             # (H, N, D)
    z = arr[..., D].astype(np.float64)                # (H, N)

    # memory-slot attention (4 keys, no bias, mask=True) on host
    mk, mv = memory_kv[0], memory_kv[1]               # (H, 4, D)
    sim_m = np.einsum('hnd,hmd->hnm', qs, mk, dtype=np.float64)
    pm = np.exp(SOFTCLAMP * np.tanh(sim_m / SOFTCLAMP))
    num = num + np.einsum('hnm,hmd->hnd', pm, mv.astype(np.float64))
    z = z + pm.sum(-1)

    out = (num / z[..., None]).astype(np.float32)[None]   # (1, H, N, D)
    if _ret_res:
        return out, res
    return out



# revision 20
# speedup vs baseline: 539.3072x; 539.3072x over previous
"""Windowed sparse attention kernel for TRN2 (8 NeuronCores).

Problem: b=1, h=16, n=16384, d=32, window w=128, nw=128 windows.
Each window of 128 queries attends to [4 memory slots | prev window | cur window]
with additive bias, tanh softcap (50), softmax.

Sharding: sequence-parallel over windows. Core c handles windows
[c*16, (c+1)*16) for all 16 heads, with a one-window k/v halo.

The graded metric is the on-device NEFF span, which is ACT-bound (tanh+exp
over 16 heads x 4096 sim columns at 1 elem/cycle/lane). The kernel keeps
every other engine under ACT's budget and minimizes per-op overheads:

- All inputs ship as fp16 in one flat [128, 29968] tensor per core (q/k/v
  scaled on host; fp16 halves DMA bytes and runs matmuls at full PE rate).
- Sim layout is task-major, no filler columns: task t owns 256 cols
  [prev(k_{t-1}) | cur(k_t)], 4096 cols/head. mm1 computes it per key-slot
  (slot s serves task s as prev and task s-1 as cur).
- DVE adds bias while evacuating PSUM (the only engine that can).
- ACT processes TWO heads per instruction ([128, 8192] tiles), tanh
  in-place then exp into fp16, amortizing per-op overhead and halving
  semaphore traffic on the critical engine.
- mm2 (p @ v~) accumulates per task into PSUM; the ones column of v~
  makes col 32 the softmax denominator Z; results DMA straight from PSUM
  to DRAM (f32), keeping DVE off the critical path.
- Host combines the 4-slot memory attention (1.5% of keys) and normalizes.
"""

import os
import numpy as np

B, H, N, D = 1, 16, 16384, 32
W = 128                 # window size
NW = N // W             # 128 windows
NCORES = 8
WPC = NW // NCORES      # 16 windows (tasks) per core
NSLOT = WPC + 1         # 17 k/v slots (halo)
SOFTCLAMP = 50.0
SCALE = D ** -0.5
MASK_PEN = -50.0        # exp(50*tanh(-50/50)) ~ e-38: dead key
SIMW = WPC * 2 * W      # 4096 sim cols per head, task-major

Q_OFF = 0               # 4 groups x 2048 cols
K_OFF = Q_OFF + 4 * WPC * W          # 8192; 4 groups x 2176 cols
V_OFF = K_OFF + 4 * NSLOT * W        # 16896; 16 heads x 561 cols
B_OFF = V_OFF + H * NSLOT * 33       # 25872; 4096 cols fp16
XCOLS = B_OFF + SIMW                 # 29968

NCORES_RUN = int(os.environ.get("BASS_ATT_CORES", "8"))
NGRP = NW // WPC // NCORES_RUN

_COMPILED = None


def _build_bass():
    import concourse.bacc as bacc
    import concourse.tile as tile
    from concourse import mybir
    from contextlib import ExitStack

    f32 = mybir.dt.float32
    f16 = mybir.dt.float16
    nc = bacc.Bacc()

    x = nc.declare_dram_parameter("x", [NGRP, 128, XCOLS], f16, isOutput=False)
    o = nc.declare_dram_parameter("o", [NGRP * H, 128, WPC * 33], f32, isOutput=True)

    with ExitStack() as ctx:
        tc = ctx.enter_context(tile.TileContext(nc))
        bias_pool = ctx.enter_context(tc.tile_pool(name="biasp", bufs=2))
        qk_pool = ctx.enter_context(tc.tile_pool(name="qk", bufs=2))
        v_pool = ctx.enter_context(tc.tile_pool(name="v", bufs=2))
        wide = ctx.enter_context(tc.tile_pool(name="wide", bufs=2))
        ow_pool = ctx.enter_context(tc.tile_pool(name="ow", bufs=2))
        sim_ps = ctx.enter_context(tc.tile_pool(name="simps", bufs=4, space="PSUM"))
        out_ps = ctx.enter_context(tc.tile_pool(name="outps", bufs=4, space="PSUM"))

        for wg in range(NGRP):
            xg = x[wg]
            biasS = bias_pool.tile([128, SIMW], f16)
            nc.sync.dma_start(out=biasS[:, :], in_=xg[:, B_OFF:B_OFF + SIMW])

            for g in range(4):
                Qg = qk_pool.tile([128, WPC * W], f16, tag="qg")
                nc.sync.dma_start(out=Qg[:, :],
                                  in_=xg[:, Q_OFF + g * WPC * W:Q_OFF + (g + 1) * WPC * W])
                Kg = qk_pool.tile([128, NSLOT * W], f16, tag="kg")
                nc.sync.dma_start(out=Kg[:, :],
                                  in_=xg[:, K_OFF + g * NSLOT * W:K_OFF + (g + 1) * NSLOT * W])
                for j in range(2):          # pairs of heads share wide tiles
                    simS = wide.tile([128, 2 * SIMW], f32, tag="simS")
                    pS = wide.tile([128, 2 * SIMW], f16, tag="pS")
                    Vp = []
                    for i2 in range(2):
                        i = 2 * j + i2
                        h = 4 * g + i
                        p0 = 32 * i
                        c0 = i2 * SIMW      # this head's cols in the pair tile
                        Vh = v_pool.tile([128, NSLOT * 33], f16)
                        nc.sync.dma_start(
                            out=Vh[:, :],
                            in_=xg[:, V_OFF + h * NSLOT * 33:V_OFF + (h + 1) * NSLOT * 33])
                        Vp.append(Vh)
                        # mm1, task-major: bank b = tasks {2b, 2b+1}, 512 cols.
                        # prev(t) = slot t keys, cur(t) = slot t+1 keys.
                        for b in range(8):
                            t0 = 2 * b
                            simP = sim_ps.tile([128, 512], f32)
                            nc.tensor.matmul(simP[:, 0:128],
                                             lhsT=Kg[p0:p0 + 32, t0 * W:(t0 + 1) * W],
                                             rhs=Qg[p0:p0 + 32, t0 * W:(t0 + 1) * W],
                                             start=True, stop=True,
                                             tile_position=(p0, 0))
                            nc.tensor.matmul(simP[:, 128:384],
                                             lhsT=Kg[p0:p0 + 32, (t0 + 1) * W:(t0 + 2) * W],
                                             rhs=Qg[p0:p0 + 32, t0 * W:(t0 + 2) * W],
                                             start=True, stop=True,
                                             tile_position=(p0, 0))
                            nc.tensor.matmul(simP[:, 384:512],
                                             lhsT=Kg[p0:p0 + 32, (t0 + 2) * W:(t0 + 3) * W],
                                             rhs=Qg[p0:p0 + 32, (t0 + 1) * W:(t0 + 2) * W],
                                             start=True, stop=True,
                                             tile_position=(p0, 0))
                            nc.vector.tensor_add(
                                simS[:, c0 + b * 512:c0 + (b + 1) * 512],
                                simP[:, :],
                                biasS[:, b * 512:(b + 1) * 512],
                            )
                    # softcap + exp, two heads per instruction
                    nc.scalar.activation(simS[:, :], simS[:, :],
                                         mybir.ActivationFunctionType.Tanh,
                                         scale=1.0 / SOFTCLAMP)
                    nc.scalar.activation(pS[:, :], simS[:, :],
                                         mybir.ActivationFunctionType.Exp,
                                         scale=SOFTCLAMP)
                    # mm2: out (128 q, 33) per task, 8 tasks per PSUM bank,
                    # DMA'd straight from PSUM to DRAM (f32).
                    for i2 in range(2):
                        i = 2 * j + i2
                        h = 4 * g + i
                        c0 = i2 * SIMW
                        Vh = Vp[i2]
                        outW = ow_pool.tile([128, WPC * 33], f32)
                        for tb in range(2):
                            otP = out_ps.tile([128, 8 * 33], f32)
                            for u in range(8):
                                t = 8 * tb + u
                                nc.tensor.matmul(
                                    otP[:, u * 33:(u + 1) * 33],
                                    lhsT=pS[:, c0 + t * 256:c0 + t * 256 + 128],
                                    rhs=Vh[:, t * 33:(t + 1) * 33],
                                    start=True, stop=False)
                                nc.tensor.matmul(
                                    otP[:, u * 33:(u + 1) * 33],
                                    lhsT=pS[:, c0 + t * 256 + 128:c0 + (t + 1) * 256],
                                    rhs=Vh[:, (t + 1) * 33:(t + 2) * 33],
                                    start=False, stop=True)
                            nc.vector.tensor_copy(
                                outW[:, tb * 264:(tb + 1) * 264], otP[:, :])
                        nc.sync.dma_start(out=o[wg * H + h], in_=outW[:, :])
    nc.compile()
    return nc


def _get_compiled():
    global _COMPILED
    if _COMPILED is None:
        _COMPILED = _build_bass()
    return _COMPILED


def _prep_core(c, qs, ks, vs, ab, mvec):
    """Build the flat per-core fp16 input. qs,ks,vs: (H, N, D) (qs pre-scaled)."""
    w0 = c * WPC
    qw = qs.reshape(H, NW, W, D)[:, w0:w0 + WPC]          # (H,16,128,32)
    qTc = qw.reshape(4, 4, WPC, W, D).transpose(0, 1, 4, 2, 3).reshape(4, 128, WPC * W)

    kw = ks.reshape(H, NW, W, D)
    vw = vs.reshape(H, NW, W, D)
    khalo = np.zeros((H, NSLOT, W, D), np.float32)
    vhalo = np.zeros((H, NSLOT, W, D), np.float32)
    lo = w0 - 1
    src_lo = max(lo, 0)
    dst_lo = src_lo - lo
    khalo[:, dst_lo:] = kw[:, src_lo:w0 + WPC]
    vhalo[:, dst_lo:] = vw[:, src_lo:w0 + WPC]
    kTc = khalo.reshape(4, 4, NSLOT, W, D).transpose(0, 1, 4, 2, 3).reshape(4, 128, NSLOT * W)
    vvc = np.concatenate([vhalo, np.ones((H, NSLOT, W, 1), np.float32)], axis=3)
    vvc = vvc.transpose(0, 2, 1, 3).reshape(H, 128, NSLOT * 33)

    # bias, task-major: task t cols [t*256, (t+1)*256) = [prev | cur], both
    # halves transposed to (key, q); key mask (+ structural masking of
    # window -1) folded as additive penalty.
    bTc = np.zeros((128, WPC, 2, W), np.float32)           # (key, task, role, q)
    for t in range(WPC):
        gw = w0 + t
        bTc[:, t, 0, :] = ab[gw, :, 0:128].T               # prev role
        bTc[:, t, 1, :] = ab[gw, :, 128:256].T             # cur role
        if gw == 0:
            pen_prev = np.full((W,), MASK_PEN, np.float32)
        else:
            pen_prev = np.where(mvec[(gw - 1) * W:gw * W], np.float32(0),
                                np.float32(MASK_PEN))
        pen_cur = np.where(mvec[gw * W:(gw + 1) * W], np.float32(0),
                           np.float32(MASK_PEN))
        bTc[:, t, 0, :] += pen_prev[:, None]
        bTc[:, t, 1, :] += pen_cur[:, None]
    bTc = bTc.reshape(128, SIMW)

    X = np.concatenate(
        [np.concatenate(list(qTc), axis=1),
         np.concatenate(list(kTc), axis=1),
         np.concatenate(list(vvc), axis=1),
         bTc], axis=1).astype(np.float16)
    assert X.shape == (128, XCOLS)
    return X


def _run_device(in_maps, trace=False):
    from concourse.bass_utils import run_bass_kernel_spmd
    nc = _get_compiled()
    res = run_bass_kernel_spmd(nc, in_maps, list(range(NCORES_RUN)), trace=trace)
    return res


def kernel(q, k, v, mask, attn_bias, memory_kv, _trace=False, _ret_res=False):
    q = np.asarray(q, np.float32)
    k = np.asarray(k, np.float32)
    v = np.asarray(v, np.float32)
    mask = np.asarray(mask)
    attn_bias = np.asarray(attn_bias, np.float32)
    memory_kv = np.asarray(memory_kv, np.float32)

    qs = q[0] * np.float32(SCALE)       # (H, N, D)
    ks, vs = k[0], v[0]
    ab = attn_bias[0]                   # (NW, W, 2W)
    mvec = mask[0].astype(bool)         # (N,)

    blocks = [_prep_core(gi, qs, ks, vs, ab, mvec) for gi in range(NW // WPC)]
    in_maps = [
        {"x": np.ascontiguousarray(np.stack(blocks[c * NGRP:(c + 1) * NGRP]))}
        for c in range(NCORES_RUN)
    ]
    res = _run_device(in_maps, trace=_trace)
    big = np.concatenate(
        [np.asarray(r["o"], dtype=np.float32) for r in res.results])

    # (group, h, q, task, 33) -> (h, group, task, q, 33) -> (h, n, 33)
    arr = big.reshape(NW // WPC, H, W, WPC, 33).transpose(1, 0, 3, 2, 4)
    arr = arr.reshape(H, N, 33)
    num = arr[..., :D].astype(np.float64)             # (H, N, D)
    z = arr[..., D].astype(np.float64)                # (H, N)

    # memory-slot attention (4 keys, no bias, mask=True) on host
    mk, mv = memory_kv[0], memory_kv[1]               # (H, 4, D)
    sim_m = np.einsum('hnd,hmd->hnm', qs, mk, dtype=np.float64)
    pm = np.exp(SOFTCLAMP * np.tanh(sim_m / SOFTCLAMP))
    num = num + np.einsum('hnm,hmd->hnd', pm, mv.astype(np.float64))
    z = z + pm.sum(-1)

    out = (num / z[..., None]).astype(np.float32)[None]   # (1, H, N, D)
    if _ret_res:
        return out, res
    return out
